# revision 13
# baseline (speedup 1.0000x reference)
"""GQA attention (B=2, T=2048, C=2048, 16Q/4KV heads, RoPE + QK-RMSNorm, causal)
as a Bass/Tile kernel on 8 NeuronCores.

Sharding: 4-way tensor-parallel over KV groups (each group = 1 KV head + its 4
query heads) x 2-way data-parallel over batch. Each core computes a partial
output projection (its 512 y-channels x full w_proj columns); the host sums the
4 TP partials per batch element.

Dtypes: all matmul operands are bf16 (PE runs 1 cyc/row), accumulation is fp32
in PSUM. RoPE/RMSNorm arithmetic, softmax row-sums and the output partials
stay fp32.

Device-side layouts (no on-device transposes needed except V):
  xT   [C, T]    x transposed (contraction dim on partitions for QKV matmul)
  wT   [C, 768]  per-core QKV weight slice, transposed; cols 0-511 = 4 q heads,
                 512-639 = k head, 640-767 = v head. Q/K head rows are
                 de-interleaved (evens then odds) so RoPE pairs sit in
                 partition blocks [0:64] / [64:128]. Applied identically to q
                 and k, this permutation leaves q.k dot products unchanged.
  wpT  [512, C]  w_proj[:, group_cols].T
  cosT/sinT [128, T] RoPE tables transposed, duplicated across both halves.
  nw   [1, 128]  q_norm_w * k_norm_w (permuted): both norm weights enter the
                 scores multiplicatively per-d, so they are folded into the
                 q-side rstd broadcast.
  outT [C, T]    fp32 partial output, transposed; host sums the 4 TP ranks and
                 transposes back.
"""

import numpy as np
import ml_dtypes

from concourse import bacc, bass, mybir
from concourse.bass_utils import run_bass_kernel_spmd
from concourse.masks import make_identity
from concourse.tile import TileContext

F32 = mybir.dt.float32
BF16 = mybir.dt.bfloat16
AF = mybir.ActivationFunctionType

B, T, C = 2, 2048, 2048
HD = 128               # head dim
QH = 4                 # q heads per core
OC = 768               # qkv out channels per core (4*128 q + 128 k + 128 v)
CT = C // 128          # 16 contraction tiles
TB = T // 512          # 4 t-blocks of 512
EPS = 1e-6


def build_program():
    nc = bacc.Bacc()
    xT = nc.declare_dram_parameter("xT", [C, T], BF16, isOutput=False)
    wT = nc.declare_dram_parameter("wT", [C, OC], BF16, isOutput=False)
    wpT = nc.declare_dram_parameter("wpT", [512, C], BF16, isOutput=False)
    cosT = nc.declare_dram_parameter("cosT", [128, T], F32, isOutput=False)
    sinT = nc.declare_dram_parameter("sinT", [128, T], F32, isOutput=False)
    nw = nc.declare_dram_parameter("nw", [1, HD], BF16, isOutput=False)
    outT = nc.declare_dram_parameter("outT", [C, T], F32, isOutput=True)

    with TileContext(nc) as tc:
        with tc.tile_pool(name="persist", bufs=1) as pp:
            w_sb = pp.tile([128, CT * OC], BF16)     # 24 KB/part
            cos_sb = pp.tile([128, T], F32)          # halves hold same rows
            sin_sb = pp.tile([128, T], F32)
            nw_sb = pp.tile([1, HD], BF16)
            ones_cf = pp.tile([128, 1], F32)
            ones_cb = pp.tile([128, 1], BF16)
            ones_rb = pp.tile([1, 128], BF16)
            ident = pp.tile([128, 128], BF16)
            epsq = pp.tile([1, 1], F32)
            epsk = pp.tile([1, 1], F32)
            krs_col = pp.tile([128, CT], F32)  # rstd_k/sqrt(hd), col i = block i
            Q = [pp.tile([128, T], F32, name=f"Q{h}") for h in range(QH)]
            Qb = [pp.tile([128, T], BF16, name=f"Qb{h}") for h in range(QH)]
            K = pp.tile([128, T], F32)
            Kb = pp.tile([128, T], BF16)
            VTb = pp.tile([128, T], BF16)
            V = pp.tile([128, T], BF16)
            Y = [pp.tile([128, T], BF16, name=f"Y{h}") for h in range(QH)]

            nc.sync.dma_start(cos_sb[:], cosT[:])
            nc.sync.dma_start(sin_sb[:], sinT[:])
            nc.sync.dma_start(nw_sb[:], nw[:])
            for ct in range(CT):
                nc.sync.dma_start(
                    w_sb[:, ct * OC:(ct + 1) * OC],
                    wT[ct * 128:(ct + 1) * 128, :],
                )
            nc.gpsimd.memset(ones_cf[:], 1.0)
            nc.gpsimd.memset(ones_cb[:], 1.0)
            nc.gpsimd.memset(ones_rb[:], 1.0)
            nc.gpsimd.memset(epsq[:], float(EPS))
            nc.gpsimd.memset(epsk[:], float(HD * EPS))
            make_identity(nc, ident[:])

            # ---------------- Phase A: QKV projection ----------------
            with (
                tc.tile_pool(name="pA_sb", bufs=1) as pa,
                tc.tile_pool(name="pA_ps", bufs=1, space="PSUM") as paps,
            ):
                for tb in range(TB):
                    ts = slice(tb * 512, (tb + 1) * 512)
                    ps = [
                        paps.tile([128, 512], F32, name=f"qkvps{j}",
                                  tag="qkv", bufs=6)
                        for j in range(6)
                    ]
                    # one big load per t-block into its own buffer: DMA slots
                    # are never reused, so the DMA instructions need no sync
                    # waits (walrus allows at most one per DMA).
                    xt = pa.tile([128, CT * 512], BF16, name="xt", tag="xt",
                                 bufs=4)
                    nc.sync.dma_start(
                        xt.rearrange("p (a t) -> p a t", a=CT),
                        xT[:, ts].rearrange("(a p) t -> p a t", p=128))
                    for ct in range(CT):
                        wof = ct * OC
                        xs = slice(ct * 512, (ct + 1) * 512)
                        for j in range(6):
                            nc.tensor.matmul(
                                ps[j][:],
                                lhsT=w_sb[:, wof + j * 128:wof + (j + 1) * 128],
                                rhs=xt[:, xs],
                                start=(ct == 0),
                                stop=(ct == CT - 1),
                            )
                    for h in range(QH):
                        nc.scalar.copy(Q[h][:, ts], ps[h][:])
                    nc.scalar.copy(K[:, ts], ps[4][:])
                    nc.scalar.copy(VTb[:, ts], ps[5][:])

                # V^T [d, t] -> V [t, d] via PE transpose, 128x128 chunks
                for i in range(CT):
                    cs = slice(i * 128, (i + 1) * 128)
                    vtr = paps.tile([128, 128], BF16, name="vtr", tag="vtr",
                                    bufs=2)
                    nc.tensor.transpose(vtr[:], VTb[:, cs], ident[:])
                    nc.scalar.copy(V[:, cs], vtr[:])

            # ---------------- Phase B: RMSNorm + RoPE ----------------
            # K first (attention depends on it), then q heads.
            with (
                tc.tile_pool(name="pN_sb", bufs=1) as pn,
                tc.tile_pool(name="pN_ps", bufs=1, space="PSUM") as pnps,
            ):
                def rms_rinv(src, is_k):
                    """returns list of 4 rinv [1,512] bf16 tiles (per t-block)"""
                    rinvs = []
                    for tb in range(TB):
                        ts = slice(tb * 512, (tb + 1) * 512)
                        sq = pn.tile([128, 512], BF16, name="sq", tag="sq",
                                     bufs=2)
                        nc.scalar.square(sq[:], src[:, ts])
                        ss = pnps.tile([1, 512], F32, name="ss", tag="ss",
                                       bufs=1)
                        nc.tensor.matmul(ss[:], lhsT=ones_cb[:], rhs=sq[:],
                                         start=True, stop=True)
                        rms = pn.tile([1, 512], F32, name="rms", tag="rms",
                                      bufs=2)
                        if is_k:
                            # folded exp scale: rstd_k/sqrt(hd)
                            #   = 1/sqrt(ss + hd*eps)
                            nc.scalar.activation(rms[:], ss[:], AF.Sqrt,
                                                 bias=epsk[:], scale=1.0)
                        else:
                            nc.scalar.activation(rms[:], ss[:], AF.Sqrt,
                                                 bias=epsq[:],
                                                 scale=float(1.0 / HD))
                        rinv = pn.tile([1, 512], BF16, name="rinv", tag="rinv",
                                       bufs=4)
                        with nc.allow_low_precision(reason="bf16 rstd feed"):
                            nc.vector.reciprocal(rinv[:], rms[:])
                        rinvs.append(rinv)
                    return rinvs

                def rope(src, dst):
                    """RoPE src (fp32, de-interleaved halves) -> dst.
                    Walrus requires both SBUF inputs of a tensor_tensor op to
                    share a start partition; outputs may differ, so each
                    cross-half product lands in the opposite half of tmp."""
                    a, b = src[0:64, :], src[64:128, :]
                    tmp = pn.tile([128, T], F32, name="tmp", tag="ropetmp",
                                  bufs=2)
                    nc.gpsimd.tensor_mul(tmp[0:64, :], b, sin_sb[64:128, :])
                    nc.gpsimd.tensor_mul(tmp[64:128, :], a, sin_sb[0:64, :])
                    nc.vector.tensor_mul(a, a, cos_sb[0:64, :])
                    nc.vector.tensor_sub(dst[0:64, :], a, tmp[0:64, :])
                    nc.vector.tensor_mul(b, b, cos_sb[64:128, :])
                    nc.vector.tensor_add(dst[64:128, :], b, tmp[64:128, :])

                # --- k head: rope -> Kb (bf16); rstd_k folded into exp scale
                k_rinvs = rms_rinv(K, is_k=True)
                rope(K, Kb)
                # transpose rstd_k row [1, T] into columns [128, CT]
                for i in range(CT):
                    tb, off = divmod(i * 128, 512)
                    kc = pnps.tile([128, 1], F32, name="kc", tag="kc", bufs=2)
                    nc.tensor.matmul(
                        kc[:], lhsT=k_rinvs[tb][:, off:off + 128],
                        rhs=ones_rb[:, 0:1], start=True, stop=True)
                    nc.scalar.copy(krs_col[:, i:i + 1], kc[:])

                # --- q heads: rope in place, then *= outer(nw, rstd_q) -> Qb
                for h in range(QH):
                    q_rinvs = rms_rinv(Q[h], is_k=False)
                    rope(Q[h], Q[h])
                    for tb in range(TB):
                        ts = slice(tb * 512, (tb + 1) * 512)
                        bc = pnps.tile([128, 512], F32, name="bc", tag="bc",
                                       bufs=2)
                        nc.tensor.matmul(bc[:], lhsT=nw_sb[:],
                                         rhs=q_rinvs[tb][:],
                                         start=True, stop=True)
                        nc.vector.tensor_mul(Qb[h][:, ts], Q[h][:, ts], bc[:])

            # ---------------- Phase C: causal attention ----------------
            with (
                tc.tile_pool(name="pC_sb", bufs=1) as pc,
                tc.tile_pool(name="pC_ps", bufs=1, space="PSUM") as pcps,
            ):
                for h in range(QH):
                    for qt in range(TB):
                        qs = slice(qt * 512, (qt + 1) * 512)
                        nkb = 4 * (qt + 1)
                        ops = pcps.tile([128, 512], F32, name="ops", tag="O",
                                        bufs=2)
                        A = pc.tile([128, 512], F32, name="A", tag="A", bufs=2)
                        for kb in range(nkb):
                            ks = slice(kb * 128, (kb + 1) * 128)
                            sps = pcps.tile([128, 512], F32, name="sps",
                                            tag="S", bufs=2)
                            nc.tensor.matmul(
                                sps[:], lhsT=Kb[:, ks], rhs=Qb[h][:, qs],
                                start=True, stop=True)
                            E = pc.tile([128, 512], BF16, name="E", tag="E",
                                        bufs=3)
                            # exp(S^T * rstd_q*rstd_k/sqrt(hd)); q factor is
                            # already in Qb, k factor is per-partition scale
                            nc.scalar.activation(E[:], sps[:], AF.Exp,
                                                 scale=krs_col[:, kb:kb + 1])
                            o = kb - 4 * qt
                            if o >= 0:
                                # causal: keep where col >= row + 128*o
                                nc.gpsimd.affine_select(
                                    E[:], E[:],
                                    pattern=[[1, 512]],
                                    compare_op=mybir.AluOpType.is_ge,
                                    fill=0.0,
                                    base=-128 * o,
                                    channel_multiplier=-1,
                                )
                            if kb == 0:
                                nc.vector.tensor_copy(A[:], E[:])
                            else:
                                nc.vector.tensor_add(A[:], A[:], E[:])
                            nc.tensor.matmul(
                                ops[:], lhsT=V[:, ks], rhs=E[:],
                                start=(kb == 0), stop=(kb == nkb - 1))
                        rs = pcps.tile([1, 512], F32, name="rs", tag="rs",
                                       bufs=1)
                        nc.tensor.matmul(rs[:], lhsT=ones_cf[:], rhs=A[:],
                                         start=True, stop=True)
                        rinv2 = pc.tile([1, 512], BF16, name="rinv2",
                                        tag="rinv2", bufs=2)
                        with nc.allow_low_precision(reason="bf16 softmax denom"):
                            nc.vector.reciprocal(rinv2[:], rs[:])
                        bc2 = pcps.tile([128, 512], F32, name="bc2", tag="bc2",
                                        bufs=1)
                        nc.tensor.matmul(bc2[:], lhsT=ones_rb[:],
                                         rhs=rinv2[:], start=True, stop=True)
                        bc2s = pc.tile([128, 512], F32, name="bc2s",
                                       tag="bc2s", bufs=2)
                        nc.scalar.copy(bc2s[:], bc2[:])
                        nc.vector.tensor_mul(Y[h][:, qs], ops[:], bc2s[:])

            # ---------------- Phase D: output projection ----------------
            with (
                tc.tile_pool(name="pD_sb", bufs=1) as pd,
                tc.tile_pool(name="pD_ps", bufs=1, space="PSUM") as pdps,
            ):
                wp_sb = pd.tile([128, 4 * C], BF16)  # 16 KB/part
                # engine-side touch first: absorbs pool-boundary deps so the
                # wp DMAs keep a single sync wait
                nc.gpsimd.memset(wp_sb[:], 0.0)
                for ci in range(4):
                    nc.sync.dma_start(
                        wp_sb[:, ci * C:(ci + 1) * C],
                        wpT[ci * 128:(ci + 1) * 128, :],
                    )
                for tb in range(TB):
                    ts = slice(tb * 512, (tb + 1) * 512)
                    for co in range(CT):
                        pps = pdps.tile([128, 512], F32, name="pps",
                                        tag="proj", bufs=4)
                        for ci in range(4):
                            nc.tensor.matmul(
                                pps[:],
                                lhsT=wp_sb[:, ci * C + co * 128:
                                           ci * C + (co + 1) * 128],
                                rhs=Y[ci][:, ts],
                                start=(ci == 0),
                                stop=(ci == 3),
                            )
                        osb = pd.tile([128, 512], F32, name="osb", tag="osb",
                                      bufs=4)
                        if co % 2 == 0:
                            nc.scalar.copy(osb[:], pps[:])
                        else:
                            nc.vector.tensor_copy(osb[:], pps[:])
                        nc.sync.dma_start(
                            outT[co * 128:(co + 1) * 128, ts], osb[:])

    nc.finalize()
    return nc


_PERM = np.concatenate([np.arange(0, 128, 2), np.arange(1, 128, 2)])


def _bf(a):
    return np.ascontiguousarray(a).astype(ml_dtypes.bfloat16)


def shard_inputs(x, w_qkv, w_proj, q_norm_w, k_norm_w, freqs_cos, freqs_sin):
    """Returns in_maps for 8 cores; core = b*4 + g."""
    cosT = np.ascontiguousarray(np.asarray(freqs_cos).T).astype(np.float32)
    sinT = np.ascontiguousarray(np.asarray(freqs_sin).T).astype(np.float32)
    cosT = np.vstack([cosT, cosT])
    sinT = np.vstack([sinT, sinT])
    nw = _bf((np.asarray(q_norm_w)[_PERM] *
              np.asarray(k_norm_w)[_PERM]).reshape(1, HD))
    x = np.asarray(x)
    w_qkv = np.asarray(w_qkv)
    w_proj = np.asarray(w_proj)
    xTs = [_bf(x[b].T) for b in range(B)]
    wTs, wpTs = [], []
    for g in range(4):
        blocks = []
        for hh in range(QH):  # q heads 4g..4g+3, de-interleaved rows
            rows = w_qkv[(4 * g + hh) * HD:(4 * g + hh + 1) * HD]
            blocks.append(rows[_PERM])
        blocks.append(w_qkv[2048 + g * HD:2048 + (g + 1) * HD][_PERM])
        blocks.append(w_qkv[2560 + g * HD:2560 + (g + 1) * HD])
        wTs.append(_bf(np.concatenate(blocks, axis=0).T))
        wpTs.append(_bf(w_proj[:, g * 512:(g + 1) * 512].T))
    in_maps = []
    for b in range(B):
        for g in range(4):
            in_maps.append({
                "xT": xTs[b],
                "wT": wTs[g],
                "wpT": wpTs[g],
                "cosT": cosT,
                "sinT": sinT,
                "nw": nw,
            })
    return in_maps


def unshard_output(results):
    """results: list of 8 dicts with 'outT' [C, T]. Returns [B, T, C]."""
    out = np.empty((B, T, C), dtype=np.float32)
    for b in range(B):
        acc = np.asarray(results[b * 4]["outT"], dtype=np.float32)
        for g in range(1, 4):
            acc = acc + np.asarray(results[b * 4 + g]["outT"])
        out[b] = acc.T
    return out


_NC = None


def kernel(x, w_qkv, w_proj, q_norm_w, k_norm_w, freqs_cos, freqs_sin):
    global _NC
    if _NC is None:
        _NC = build_program()
    in_maps = shard_inputs(x, w_qkv, w_proj, q_norm_w, k_norm_w,
                           freqs_cos, freqs_sin)
    res = run_bass_kernel_spmd(_NC, in_maps, list(range(8)))
    return unshard_output(res.results)


if __name__ == "__main__":
    nc = build_program()
    print("program built ok")
